# revision 25
# baseline (speedup 1.0000x reference)
"""Trainium2 Bass kernel for nn_AttentionModel (B=4, C=128, H=W=64).

Self-attention over spatial positions with 1x1-conv QKV projections and a
gamma-scaled residual:
    out = gamma * softmax(Q K / sqrt(C)) V + x

Sharding: data-parallel over batch (4 samples) x sequence-parallel over
query rows (2 halves of N=4096) = 8 NeuronCores. Each core holds the full
[C,C] weights, computes K/V for its whole sample, and the attention output
for its 2048 query rows.

Per-core algorithm (matmuls in bf16 with fp32 PSUM accumulate):
  QT[c,n] = WqT.T @ xf (+bq)   (pre-scaled by 1/sqrt(C) on host)
  K [c,m] = WkT.T @ xf (+bk)
  V [m,c] = xf_chunk.T @ WvT    (32 chunks of 128 rows; bv folded at end)
  per 1024-wide supergroup of query rows n, per 128-chunk of key index m:
    S^T[m,n] = K_chunk.T @ QT     (PE, 2 matmuls into a 2-bank PSUM tile)
    P^T      = exp(S^T)           (ACT, one op per [128,1024], bf16 out)
    acc     += P^T                (DVE, bf16 partial row-sums per lane)
    pvacc   += V_chunk.T @ P^T    (PE, PSUM accumulate)
  rowsum = ones.T @ acc           (PE: fp32 reduction of the 128 lanes)
  out = (gamma*pvacc) * recip_approx(rowsum) + gamma*bv + x   (DVE)

Scheduling notes: the kernel is ACT(exp)-bound at ~1.1us per [128,1024]
unit; everything else (PE matmuls, DVE row-sum adds, DMA) hides under the
exp stream. Per-core x is pre-rotated on the host so the 2048 query
columns sit at 0:2048 (the key/value index m is a pure reduction index,
so a permutation is harmless) -- this makes the query slice a view of xf
and lets a small [C,1024] "head" DMA gate the first projections while the
bulk transfers are WAW-fenced behind it. Row-sum adds stay OFF gpsimd
(its port sharing with the DVE slows both), and exp skips the usual
max-subtraction: energies here are ~N(0,1), safely inside exp's range.
"""

import numpy as np
import ml_dtypes

import concourse.bass as bass
import concourse.mybir as mybir
import concourse.tile as tile
from concourse import bacc
from concourse.bass_utils import run_bass_kernel_spmd

B, C, H, W = 4, 128, 64, 64
N = H * W            # 4096 spatial positions
NCORES = 8
RQ = N * B // NCORES  # 2048 query rows per core
NG = 512             # query-row group width (PSUM bank)
MC = 128             # key-chunk width (PE contraction)
F32 = mybir.dt.float32
BF16 = mybir.dt.bfloat16
AF = mybir.ActivationFunctionType


def build_bass():
    nc = bacc.Bacc("TRN2", target_bir_lowering=False, debug=False,
                   num_devices=NCORES)

    xf = nc.dram_tensor("xf", [C, N], BF16, kind="ExternalInput")
    xh = nc.dram_tensor("xh", [C, 1024], BF16, kind="ExternalInput")
    xr = nc.dram_tensor("xr", [C, RQ], F32, kind="ExternalInput")
    wct = nc.dram_tensor("wct", [C, 3, C], BF16, kind="ExternalInput")
    bb = nc.dram_tensor("bb", [C, 4], F32, kind="ExternalInput")
    out = nc.dram_tensor("out", [C, RQ], F32, kind="ExternalOutput")

    n_mc = N // MC       # 32 key chunks
    NSG = 1024           # query supergroup width
    n_sg = RQ // NSG     # 2 supergroups

    with tile.TileContext(nc) as tc:
        with tc.tile_pool(name="const", bufs=1) as cp:
            xf_t = cp.tile([C, N], BF16, tag="xf")
            xh_t = cp.tile([C, 1024], BF16, tag="xh")
            xr_t = cp.tile([C, RQ], F32, tag="xr")
            wc_t = cp.tile([C, 3, C], BF16, tag="wc")
            bb_t = cp.tile([C, 4], F32, tag="bb")
            ones_t = cp.tile([C, C], BF16, tag="ones")
            kk_t = cp.tile([C, N], BF16, tag="kk")
            qt_t = cp.tile([C, RQ], BF16, tag="qt")
            vv_t = cp.tile([C, n_mc, MC], BF16, tag="vv")
            wq_t, wk_t, wv_t = wc_t[:, 0, :], wc_t[:, 1, :], wc_t[:, 2, :]
            bq_t, bk_t = bb_t[:, 0:1], bb_t[:, 1:2]
            bvg_t, gm_t = bb_t[:, 2:3], bb_t[:, 3:4]

            # Small/urgent loads on the HWDGE (sync) queue; bulk x loads on
            # the SWDGE (gpsimd) queue so the two issue streams overlap.
            warm = cp.tile([C, 1], F32, tag="warm")
            nc.gpsimd.memset(warm[:], 0.0)
            nc.scalar.activation(warm[:], warm[:], AF.Exp)
            # xf is pre-rotated per core so the query block is cols 0:2048.
            # A small duplicate head tile gates the first projections while
            # the bulk loads stream with wide (8KB/partition) packets.
            nc.sync.dma_start(xh_t[:, bass.ts(0, NG)], xh[:, bass.ts(0, NG)])
            nc.sync.dma_start(wc_t[:], wct[:])
            nc.sync.dma_start(bb_t[:], bb[:])
            nc.sync.dma_start(xh_t[:, bass.ts(1, NG)], xh[:, bass.ts(1, NG)])
            nc.vector.memset(ones_t[:], 1.0)
            # gate the bulk loads behind xh arrival: a tiny copy that READS
            # xh and WRITES the head of each bulk destination region forces a
            # WAW ordering, so the head tile gets the HBM pipe to itself.
            # (cols 0:1024 of xf are only ever read via xh.)
            for dst in (xf_t[:, 1024:1025], xf_t[:, 2560:2561],
                        xr_t[:, 0:1], xr_t[:, 1024:1025]):
                nc.vector.tensor_copy(dst, xh_t[:, 0:1])
            nc.scalar.dma_start(xf_t[:, bass.ds(1024, 1536)],
                                xf[:, bass.ds(1024, 1536)])
            nc.scalar.dma_start(xf_t[:, bass.ds(2560, 1536)],
                                xf[:, bass.ds(2560, 1536)])
            nc.gpsimd.dma_start(xr_t[:, bass.ts(0, 1024)],
                                xr[:, bass.ts(0, 1024)])
            nc.gpsimd.dma_start(xr_t[:, bass.ts(1, 1024)],
                                xr[:, bass.ts(1, 1024)])

            with (
                tc.tile_pool(name="stp", bufs=2,
                             space=bass.MemorySpace.PSUM) as stp,
                tc.tile_pool(name="pvp", bufs=1,
                             space=bass.MemorySpace.PSUM) as pvp,
                tc.tile_pool(name="vpp", bufs=2,
                             space=bass.MemorySpace.PSUM) as vpp,
                tc.tile_pool(name="ptp", bufs=14) as ptp,
                tc.tile_pool(name="accp", bufs=2) as accp,
                tc.tile_pool(name="fin", bufs=2) as fin,
            ):
                def proj(dst, w, src, bias, on_act=False):
                    ps = vpp.tile([C, NG], F32, tag="vp")
                    nc.tensor.matmul(ps[:], w, src, start=True, stop=True)
                    if on_act:
                        nc.scalar.activation(dst, ps[:], AF.Identity,
                                             bias=bias)
                    else:
                        nc.vector.tensor_scalar_add(out=dst, in0=ps[:],
                                                    scalar1=bias)

                def vbatch(mc0):
                    # V projection for key chunks mc0..mc0+3 in one PSUM
                    # tile, one PSUM->SBUF copy
                    vp = vpp.tile([C, NG], F32, tag="vp")
                    for i in range(4):
                        xsrc = xh_t if mc0 + i < 8 else xf_t
                        nc.tensor.matmul(vp[:, bass.ts(i, MC)],
                                         xsrc[:, bass.ts(mc0 + i, MC)],
                                         wv_t, start=True, stop=True)
                    nc.vector.tensor_copy(vv_t[:, mc0:mc0 + 4, :], vp[:])

                # only what the first S^T matmul needs; four PSUM tiles
                # (vpp x2 + st x2, all idle here) and both bias engines run
                # these projections fully in parallel
                def proj_st(dst, w, src, bias, on_act=False):
                    ps = stp.tile([C, NG], F32, tag="st")
                    nc.tensor.matmul(ps[:], w, src, start=True, stop=True)
                    if on_act:
                        nc.scalar.activation(dst, ps[:], AF.Identity,
                                             bias=bias)
                    else:
                        nc.vector.tensor_scalar_add(out=dst, in0=ps[:],
                                                    scalar1=bias)

                proj(qt_t[:, bass.ts(0, NG)], wq_t, xh_t[:, bass.ts(0, NG)],
                     bq_t, on_act=True)
                proj(qt_t[:, bass.ts(1, NG)], wq_t, xh_t[:, bass.ts(1, NG)],
                     bq_t, on_act=True)
                proj_st(kk_t[:, bass.ts(0, NG)], wk_t, xh_t[:, bass.ts(0, NG)],
                        bk_t)
                proj_st(kk_t[:, bass.ts(1, NG)], wk_t, xh_t[:, bass.ts(1, NG)],
                        bk_t)

                for sg in range(n_sg):
                    pv_ps = pvp.tile([C, NSG], F32, tag="pv")
                    acc_d = accp.tile([C, NSG], BF16, tag="acc_d")
                    acc_g = accp.tile([C, NSG], BF16, tag="acc_g")
                    for mc in range(n_mc):
                        st_ps = stp.tile([C, NSG], F32, tag="st")
                        for q in range(NSG // NG):
                            nn = sg * NSG + q * NG
                            nc.tensor.matmul(
                                st_ps[:, bass.ts(q, NG)],
                                kk_t[:, bass.ts(mc, MC)],
                                qt_t[:, bass.ds(nn, NG)],
                                start=True, stop=True)
                        pt = ptp.tile([C, NSG], BF16, tag="pt")
                        nc.scalar.activation(pt[:], st_ps[:], AF.Exp)
                        if sg == 0:
                            if mc == 0:
                                vbatch(0)
                            # just-in-time projections for upcoming chunks
                            if mc % 2 == 1 and mc <= 11:
                                j = 2 + (mc - 1) // 2
                                js = bass.ts(j, NG)
                                proj(kk_t[:, js], wk_t, xf_t[:, js], bk_t)
                            if mc in (13, 15):
                                j = 2 + (mc - 13) // 2
                                proj(qt_t[:, bass.ts(j, NG)], wq_t,
                                     xf_t[:, bass.ts(j, NG)], bq_t)
                            if mc % 4 == 0 and mc + 4 < n_mc:
                                vbatch(mc + 4)
                        for q in range(NSG // NG):
                            nc.tensor.matmul(
                                pv_ps[:, bass.ts(q, NG)],
                                vv_t[:, mc, :], pt[:, bass.ts(q, NG)],
                                start=(mc == 0), stop=(mc == n_mc - 1))
                        acc = acc_g if mc % 2 == 1 else acc_d
                        if mc < 2:
                            nc.vector.tensor_copy(acc[:], pt[:])
                        else:
                            nc.vector.tensor_add(acc[:], acc[:], pt[:])

                    # rowsum = ones.T @ acc_g + ones.T @ acc_d (acc_g is
                    # final after chunk 27 so its matmuls overlap the tail
                    # chunks; acc_d finishes right after the last exp)
                    # t0 = gamma * pv frees the PV accumulator before the
                    # reciprocal chain, so the next supergroup starts clean
                    rb = fin.tile([C, NSG], F32, tag="rb")
                    t0 = fin.tile([C, NSG], F32, tag="t0")
                    t1 = fin.tile([C, NSG], F32, tag="t1")
                    o3 = fin.tile([C, NSG], F32, tag="o3")
                    for q in range(NSG // NG):
                        s = bass.ts(q, NG)
                        nc.vector.tensor_scalar_mul(out=t0[:, s],
                                                    in0=pv_ps[:, s],
                                                    scalar1=gm_t)
                    for q in range(NSG // NG):
                        s = bass.ts(q, NG)
                        nn = bass.ds(sg * NSG + q * NG, NG)
                        rs_ps = vpp.tile([C, NG], F32, tag="vp")
                        nc.tensor.matmul(rs_ps[:], ones_t[:],
                                         acc_d[:, s], start=True, stop=False)
                        nc.tensor.matmul(rs_ps[:], ones_t[:],
                                         acc_g[:, s], start=False, stop=True)
                        nc.vector.reciprocal_approx_fast(out=rb[:, s],
                                                         in_=rs_ps[:])
                        nc.vector.tensor_mul(t1[:, s], t0[:, s], rb[:, s])
                        nc.vector.scalar_tensor_tensor(
                            out=o3[:, s], in0=t1[:, s], scalar=bvg_t,
                            in1=xr_t[:, nn],
                            op0=mybir.AluOpType.add, op1=mybir.AluOpType.add)
                        oeng = nc.sync if q % 2 == 0 else nc.scalar
                        oeng.dma_start(out[:, nn], o3[:, s])

    nc.compile()
    return nc


_NC_CACHE = None


def _get_nc():
    global _NC_CACHE
    if _NC_CACHE is None:
        _NC_CACHE = build_bass()
    return _NC_CACHE


def make_in_maps(x, Wq, bq, Wk, bk, Wv, bv, gamma):
    x = np.asarray(x, dtype=np.float32)
    Wq = np.asarray(Wq, dtype=np.float32)
    Wk = np.asarray(Wk, dtype=np.float32)
    Wv = np.asarray(Wv, dtype=np.float32)
    bq = np.asarray(bq, dtype=np.float32)
    bk = np.asarray(bk, dtype=np.float32)
    bv = np.asarray(bv, dtype=np.float32)
    gamma = np.asarray(gamma, dtype=np.float32)

    scale = np.float32(1.0 / np.sqrt(C))
    xf = x.reshape(B, C, N)
    wct_s = np.ascontiguousarray(
        np.stack([(Wq * scale).T, Wk.T, Wv.T], axis=1)
    ).astype(ml_dtypes.bfloat16)  # [C_in, 3, C_out]
    g0 = np.float32(gamma.reshape(-1)[0])
    bb_s = np.ascontiguousarray(
        np.stack([bq * scale, bk, bv * g0, np.full(C, g0, np.float32)],
                 axis=1)).astype(np.float32)

    in_maps = []
    for core in range(NCORES):
        b, h = core // 2, core % 2
        xrot = np.roll(xf[b], -h * RQ, axis=1)
        xrot_bf = np.ascontiguousarray(xrot).astype(ml_dtypes.bfloat16)
        in_maps.append({
            "xf": xrot_bf,
            "xh": np.ascontiguousarray(xrot_bf[:, :1024]),
            "xr": np.ascontiguousarray(xrot[:, :RQ]),
            "wct": wct_s, "bb": bb_s,
        })
    return in_maps


def assemble(results):
    out = np.empty((B, C, N), dtype=np.float32)
    for core in range(NCORES):
        b, h = core // 2, core % 2
        out[b][:, h * RQ:(h + 1) * RQ] = results[core]["out"]
    return out.reshape(B, C, H, W)


def run(inputs: dict, trace: bool = False, tmpdir: str | None = None):
    nc = _get_nc()
    in_maps = make_in_maps(**inputs)
    last_err = None
    for _ in range(3):  # the NRT occasionally reports a transient
        try:                # device-unrecoverable error; a retry clears it
            res = run_bass_kernel_spmd(nc, in_maps,
                                       core_ids=list(range(NCORES)),
                                       trace=trace, tmpdir=tmpdir)
            return assemble(res.results), res
        except Exception as e:  # noqa: BLE001
            last_err = e
    raise last_err


def kernel(**inputs) -> np.ndarray:
    out, _ = run(inputs, trace=False)
    return out


# revision 26
# speedup vs baseline: 1.0152x; 1.0152x over previous
"""Trainium2 Bass kernel for nn_AttentionModel (B=4, C=128, H=W=64).

Self-attention over spatial positions with 1x1-conv QKV projections and a
gamma-scaled residual:
    out = gamma * softmax(Q K / sqrt(C)) V + x

Sharding: data-parallel over batch (4 samples) x sequence-parallel over
query rows (2 halves of N=4096) = 8 NeuronCores. Each core holds the full
[C,C] weights, computes K/V for its whole sample, and the attention output
for its 2048 query rows.

Per-core algorithm (matmuls in bf16 with fp32 PSUM accumulate):
  QT[c,n] = WqT.T @ xf (+bq)   (pre-scaled by 1/sqrt(C) on host)
  K [c,m] = WkT.T @ xf (+bk)
  V [m,c] = xf_chunk.T @ WvT    (32 chunks of 128 rows; bv folded at end)
  per 1024-wide supergroup of query rows n, per 128-chunk of key index m:
    S^T[m,n] = K_chunk.T @ QT     (PE, 2 matmuls into a 2-bank PSUM tile)
    P^T      = exp(S^T)           (ACT, one op per [128,1024], bf16 out)
    acc     += P^T                (DVE, bf16 partial row-sums per lane)
    pvacc   += V_chunk.T @ P^T    (PE, PSUM accumulate)
  rowsum = ones.T @ acc           (PE: fp32 reduction of the 128 lanes)
  out = (gamma*pvacc) * recip_approx(rowsum) + gamma*bv + x   (DVE)

Scheduling notes: the kernel is ACT(exp)-bound at ~1.1us per [128,1024]
unit; everything else (PE matmuls, DVE row-sum adds, DMA) hides under the
exp stream. Per-core x is pre-rotated on the host so the 2048 query
columns sit at 0:2048 (the key/value index m is a pure reduction index,
so a permutation is harmless) -- this makes the query slice a view of xf
and lets a small [C,1024] "head" DMA gate the first projections while the
bulk transfers are WAW-fenced behind it. Row-sum adds stay OFF gpsimd
(its port sharing with the DVE slows both), and exp skips the usual
max-subtraction: energies here are ~N(0,1), safely inside exp's range.
"""

import numpy as np
import ml_dtypes

import concourse.bass as bass
import concourse.mybir as mybir
import concourse.tile as tile
from concourse import bacc
from concourse.bass_utils import run_bass_kernel_spmd

B, C, H, W = 4, 128, 64, 64
N = H * W            # 4096 spatial positions
NCORES = 8
RQ = N * B // NCORES  # 2048 query rows per core
NG = 512             # query-row group width (PSUM bank)
MC = 128             # key-chunk width (PE contraction)
F32 = mybir.dt.float32
BF16 = mybir.dt.bfloat16
AF = mybir.ActivationFunctionType


def build_bass():
    nc = bacc.Bacc("TRN2", target_bir_lowering=False, debug=False,
                   num_devices=NCORES)

    xf = nc.dram_tensor("xf", [C, N], BF16, kind="ExternalInput")
    xh = nc.dram_tensor("xh", [C, 1024], BF16, kind="ExternalInput")
    xr = nc.dram_tensor("xr", [C, RQ], F32, kind="ExternalInput")
    wct = nc.dram_tensor("wct", [C, 3, C], BF16, kind="ExternalInput")
    bb = nc.dram_tensor("bb", [C, 4], F32, kind="ExternalInput")
    out = nc.dram_tensor("out", [C, RQ], F32, kind="ExternalOutput")

    n_mc = N // MC       # 32 key chunks
    NSG = 1024           # query supergroup width
    n_sg = RQ // NSG     # 2 supergroups

    with tile.TileContext(nc) as tc:
        with tc.tile_pool(name="const", bufs=1) as cp:
            xf_t = cp.tile([C, N], BF16, tag="xf")
            xh_t = cp.tile([C, 1024], BF16, tag="xh")
            xr_t = cp.tile([C, RQ], F32, tag="xr")
            wc_t = cp.tile([C, 3, C], BF16, tag="wc")
            bb_t = cp.tile([C, 4], F32, tag="bb")
            ones_t = cp.tile([C, C], BF16, tag="ones")
            kk_t = cp.tile([C, N], BF16, tag="kk")
            qt_t = cp.tile([C, RQ], BF16, tag="qt")
            vv_t = cp.tile([C, n_mc, MC], BF16, tag="vv")
            wq_t, wk_t, wv_t = wc_t[:, 0, :], wc_t[:, 1, :], wc_t[:, 2, :]
            bq_t, bk_t = bb_t[:, 0:1], bb_t[:, 1:2]
            bvg_t, gm_t = bb_t[:, 2:3], bb_t[:, 3:4]

            # Small/urgent loads on the HWDGE (sync) queue; bulk x loads on
            # the SWDGE (gpsimd) queue so the two issue streams overlap.
            warm = cp.tile([C, 1], F32, tag="warm")
            nc.gpsimd.memset(warm[:], 0.0)
            nc.scalar.activation(warm[:], warm[:], AF.Exp)
            # xf is pre-rotated per core so the query block is cols 0:2048.
            # A small duplicate head tile gates the first projections while
            # the bulk loads stream with wide (8KB/partition) packets.
            nc.sync.dma_start(xh_t[:], xh[:])
            nc.sync.dma_start(wc_t[:], wct[:])
            nc.sync.dma_start(bb_t[:], bb[:])
            nc.vector.memset(ones_t[:], 1.0)
            # gate the bulk loads behind xh arrival: a tiny copy that READS
            # xh and WRITES the head of each bulk destination region forces a
            # WAW ordering, so the head tile gets the HBM pipe to itself.
            # (cols 0:1024 of xf are only ever read via xh.)
            for dst in (xf_t[:, 1024:1025], xf_t[:, 2560:2561],
                        xr_t[:, 0:1], xr_t[:, 1024:1025]):
                nc.vector.tensor_copy(dst, xh_t[:, 0:1])
            nc.scalar.dma_start(xf_t[:, bass.ds(1024, 1536)],
                                xf[:, bass.ds(1024, 1536)])
            nc.scalar.dma_start(xf_t[:, bass.ds(2560, 1536)],
                                xf[:, bass.ds(2560, 1536)])
            nc.gpsimd.dma_start(xr_t[:, bass.ts(0, 1024)],
                                xr[:, bass.ts(0, 1024)])
            nc.gpsimd.dma_start(xr_t[:, bass.ts(1, 1024)],
                                xr[:, bass.ts(1, 1024)])

            with (
                tc.tile_pool(name="stp", bufs=2,
                             space=bass.MemorySpace.PSUM) as stp,
                tc.tile_pool(name="pvp", bufs=1,
                             space=bass.MemorySpace.PSUM) as pvp,
                tc.tile_pool(name="vpp", bufs=2,
                             space=bass.MemorySpace.PSUM) as vpp,
                tc.tile_pool(name="ptp", bufs=14) as ptp,
                tc.tile_pool(name="accp", bufs=2) as accp,
                tc.tile_pool(name="fin", bufs=2) as fin,
            ):
                def proj(dst, w, src, bias, on_act=False):
                    ps = vpp.tile([C, NG], F32, tag="vp")
                    nc.tensor.matmul(ps[:], w, src, start=True, stop=True)
                    if on_act:
                        nc.scalar.activation(dst, ps[:], AF.Identity,
                                             bias=bias)
                    else:
                        nc.vector.tensor_scalar_add(out=dst, in0=ps[:],
                                                    scalar1=bias)

                def vbatch(mc0):
                    # V projection for key chunks mc0..mc0+3 in one PSUM
                    # tile, one PSUM->SBUF copy
                    vp = vpp.tile([C, NG], F32, tag="vp")
                    for i in range(4):
                        xsrc = xh_t if mc0 + i < 8 else xf_t
                        nc.tensor.matmul(vp[:, bass.ts(i, MC)],
                                         xsrc[:, bass.ts(mc0 + i, MC)],
                                         wv_t, start=True, stop=True)
                    nc.vector.tensor_copy(vv_t[:, mc0:mc0 + 4, :], vp[:])

                # only what the first S^T matmul needs; four PSUM tiles
                # (vpp x2 + st x2, all idle here) and both bias engines run
                # these projections fully in parallel
                def proj_st(dst, w, src, bias, on_act=False):
                    ps = stp.tile([C, NG], F32, tag="st")
                    nc.tensor.matmul(ps[:], w, src, start=True, stop=True)
                    if on_act:
                        nc.scalar.activation(dst, ps[:], AF.Identity,
                                             bias=bias)
                    else:
                        nc.vector.tensor_scalar_add(out=dst, in0=ps[:],
                                                    scalar1=bias)

                proj(qt_t[:, bass.ts(0, NG)], wq_t, xh_t[:, bass.ts(0, NG)],
                     bq_t, on_act=True)
                proj(qt_t[:, bass.ts(1, NG)], wq_t, xh_t[:, bass.ts(1, NG)],
                     bq_t, on_act=True)
                proj_st(kk_t[:, bass.ts(0, NG)], wk_t, xh_t[:, bass.ts(0, NG)],
                        bk_t)
                proj_st(kk_t[:, bass.ts(1, NG)], wk_t, xh_t[:, bass.ts(1, NG)],
                        bk_t)

                for sg in range(n_sg):
                    pv_ps = pvp.tile([C, NSG], F32, tag="pv")
                    acc_d = accp.tile([C, NSG], BF16, tag="acc_d")
                    acc_g = accp.tile([C, NSG], BF16, tag="acc_g")
                    for mc in range(n_mc):
                        st_ps = stp.tile([C, NSG], F32, tag="st")
                        for q in range(NSG // NG):
                            nn = sg * NSG + q * NG
                            nc.tensor.matmul(
                                st_ps[:, bass.ts(q, NG)],
                                kk_t[:, bass.ts(mc, MC)],
                                qt_t[:, bass.ds(nn, NG)],
                                start=True, stop=True)
                        pt = ptp.tile([C, NSG], BF16, tag="pt")
                        nc.scalar.activation(pt[:], st_ps[:], AF.Exp)
                        if sg == 0:
                            if mc == 0:
                                vbatch(0)
                            # just-in-time projections for upcoming chunks
                            if mc % 2 == 1 and mc <= 11:
                                j = 2 + (mc - 1) // 2
                                js = bass.ts(j, NG)
                                proj(kk_t[:, js], wk_t, xf_t[:, js], bk_t)
                            if mc in (13, 15):
                                j = 2 + (mc - 13) // 2
                                proj(qt_t[:, bass.ts(j, NG)], wq_t,
                                     xf_t[:, bass.ts(j, NG)], bq_t)
                            if mc % 4 == 0 and mc + 4 < n_mc:
                                vbatch(mc + 4)
                        for q in range(NSG // NG):
                            nc.tensor.matmul(
                                pv_ps[:, bass.ts(q, NG)],
                                vv_t[:, mc, :], pt[:, bass.ts(q, NG)],
                                start=(mc == 0), stop=(mc == n_mc - 1))
                        acc = acc_g if mc % 2 == 1 else acc_d
                        if mc < 2:
                            nc.vector.tensor_copy(acc[:], pt[:])
                        else:
                            nc.vector.tensor_add(acc[:], acc[:], pt[:])

                    # rowsum = ones.T @ acc_g + ones.T @ acc_d (acc_g is
                    # final after chunk 27 so its matmuls overlap the tail
                    # chunks; acc_d finishes right after the last exp)
                    # t0 = gamma * pv frees the PV accumulator before the
                    # reciprocal chain, so the next supergroup starts clean
                    rb = fin.tile([C, NSG], F32, tag="rb")
                    t0 = fin.tile([C, NSG], F32, tag="t0")
                    t1 = fin.tile([C, NSG], F32, tag="t1")
                    o3 = fin.tile([C, NSG], F32, tag="o3")
                    for q in range(NSG // NG):
                        s = bass.ts(q, NG)
                        nc.vector.tensor_scalar_mul(out=t0[:, s],
                                                    in0=pv_ps[:, s],
                                                    scalar1=gm_t)
                    for q in range(NSG // NG):
                        s = bass.ts(q, NG)
                        nn = bass.ds(sg * NSG + q * NG, NG)
                        rs_ps = vpp.tile([C, NG], F32, tag="vp")
                        nc.tensor.matmul(rs_ps[:], ones_t[:],
                                         acc_d[:, s], start=True, stop=False)
                        nc.tensor.matmul(rs_ps[:], ones_t[:],
                                         acc_g[:, s], start=False, stop=True)
                        nc.vector.reciprocal_approx_fast(out=rb[:, s],
                                                         in_=rs_ps[:])
                        nc.vector.tensor_mul(t1[:, s], t0[:, s], rb[:, s])
                        nc.vector.scalar_tensor_tensor(
                            out=o3[:, s], in0=t1[:, s], scalar=bvg_t,
                            in1=xr_t[:, nn],
                            op0=mybir.AluOpType.add, op1=mybir.AluOpType.add)
                        oeng = nc.sync if q % 2 == 0 else nc.scalar
                        oeng.dma_start(out[:, nn], o3[:, s])

    nc.compile()
    return nc


_NC_CACHE = None


def _get_nc():
    global _NC_CACHE
    if _NC_CACHE is None:
        _NC_CACHE = build_bass()
    return _NC_CACHE


def make_in_maps(x, Wq, bq, Wk, bk, Wv, bv, gamma):
    x = np.asarray(x, dtype=np.float32)
    Wq = np.asarray(Wq, dtype=np.float32)
    Wk = np.asarray(Wk, dtype=np.float32)
    Wv = np.asarray(Wv, dtype=np.float32)
    bq = np.asarray(bq, dtype=np.float32)
    bk = np.asarray(bk, dtype=np.float32)
    bv = np.asarray(bv, dtype=np.float32)
    gamma = np.asarray(gamma, dtype=np.float32)

    scale = np.float32(1.0 / np.sqrt(C))
    xf = x.reshape(B, C, N)
    wct_s = np.ascontiguousarray(
        np.stack([(Wq * scale).T, Wk.T, Wv.T], axis=1)
    ).astype(ml_dtypes.bfloat16)  # [C_in, 3, C_out]
    g0 = np.float32(gamma.reshape(-1)[0])
    bb_s = np.ascontiguousarray(
        np.stack([bq * scale, bk, bv * g0, np.full(C, g0, np.float32)],
                 axis=1)).astype(np.float32)

    in_maps = []
    for core in range(NCORES):
        b, h = core // 2, core % 2
        xrot = np.roll(xf[b], -h * RQ, axis=1)
        xrot_bf = np.ascontiguousarray(xrot).astype(ml_dtypes.bfloat16)
        in_maps.append({
            "xf": xrot_bf,
            "xh": np.ascontiguousarray(xrot_bf[:, :1024]),
            "xr": np.ascontiguousarray(xrot[:, :RQ]),
            "wct": wct_s, "bb": bb_s,
        })
    return in_maps


def assemble(results):
    out = np.empty((B, C, N), dtype=np.float32)
    for core in range(NCORES):
        b, h = core // 2, core % 2
        out[b][:, h * RQ:(h + 1) * RQ] = results[core]["out"]
    return out.reshape(B, C, H, W)


def run(inputs: dict, trace: bool = False, tmpdir: str | None = None):
    nc = _get_nc()
    in_maps = make_in_maps(**inputs)
    last_err = None
    for _ in range(3):  # the NRT occasionally reports a transient
        try:                # device-unrecoverable error; a retry clears it
            res = run_bass_kernel_spmd(nc, in_maps,
                                       core_ids=list(range(NCORES)),
                                       trace=trace, tmpdir=tmpdir)
            return assemble(res.results), res
        except Exception as e:  # noqa: BLE001
            last_err = e
    raise last_err


def kernel(**inputs) -> np.ndarray:
    out, _ = run(inputs, trace=False)
    return out
